# revision 1
# baseline (speedup 1.0000x reference)
"""GridAttention Trainium2 kernel.

Full inputs -> full output. Internally shards (batch, head-pair) across 8
NeuronCores: core c handles batch c//4 and heads (2*(c%4), 2*(c%4)+1).

Math notes:
 - Attention scores are computed TRANSPOSED: S^T[j, i] = k_j . q_i * scale
   + bias[i, j], laid out [k partitions, q free]. This makes softmax-exp
   elementwise, the denominator a matmul reduction (ones column in V), and
   P^T directly usable as the moving operand of the AV matmul (no P
   transpose).
 - The 2D relative-position bias decomposes as
     bias[i, j] = rowtab[ri-rj+63, h] + coltab[ci-cj+47, h]
   (no clipping needed since H==MAX_H, W==MAX_W). Each term is a rank-
   structured product: onehot(rj)^T @ RowR and onehot(cj)^T @ ColR with
   host-precomputed feature matrices, so the bias is folded into the QK
   contraction (rows 64..127 of an augmented K=128 contraction) plus one
   extra K=48 matmul per tile.
 - No max-subtraction in softmax: logits ~ N(0,1), exp is safe in fp32 and
   softmax is shift-invariant so results match the reference.
 - All matmul operands are fp16 (1 cyc/row on the PE array); PSUM
   accumulation and the softmax normalization stay fp32.
"""

import numpy as np

EMBED = 512
NH = 8
HD = 64
GH, GW = 64, 48
B = 2
S = GH * GW  # 3072
N_CORES = 8
NQ = S // 512  # 6 q chunks of 512
NM = S // 128  # 24 k chunks of 128
NT = S // 128  # 24 q tiles of 128 (final)
KC = 4  # 512 = 4 contraction chunks of 128

_CACHE = {}


def _build_program():
    import concourse.bass as bass
    import concourse.tile as tile
    import concourse.mybir as mybir
    from concourse import bacc
    from concourse.bass import ts, ds
    from concourse.masks import make_identity

    f32 = mybir.dt.float32
    bf16 = mybir.dt.float16
    EXP = mybir.ActivationFunctionType.Exp

    nc = bacc.Bacc("TRN2", target_bir_lowering=False, debug=False,
                   num_devices=N_CORES)

    def inp(name, shape):
        return nc.dram_tensor(name, shape, bf16, kind="ExternalInput").ap()

    xT_d = inp("xT", [EMBED, S])
    wq_d = inp("wq", [EMBED, 128])
    wk_d = inp("wk", [EMBED, 128])
    wv_d = inp("wv", [EMBED, 128])
    wouta_d = inp("wouta", [HD, EMBED])
    woutb_d = inp("woutb", [HD, EMBED])
    ohr_d = inp("ohr", [64, S])
    rowra_d = inp("rowra", [64, S])
    rowrb_d = inp("rowrb", [64, S])
    oc_d = inp("oc", [48, 384])
    colra_d = inp("colra", [48, S])
    colrb_d = inp("colrb", [48, S])
    out_d = nc.dram_tensor("out", [S, EMBED], f32, kind="ExternalOutput").ap()

    with tile.TileContext(nc) as tc:
        with (
            tc.tile_pool(name="const", bufs=1) as cpool,
            tc.tile_pool(name="work", bufs=4) as wpool,
            tc.tile_pool(name="ptp", bufs=3) as ptpool,
            tc.tile_pool(name="osb", bufs=2) as opool,
            tc.tile_pool(name="pst", bufs=2, space="PSUM") as pst,
            tc.tile_pool(name="pacc", bufs=2, space="PSUM") as pacc,
            tc.tile_pool(name="pmisc", bufs=2, space="PSUM") as pmisc,
        ):
            # ---- resident SBUF tensors ----
            xT = cpool.tile([128, KC * S], bf16)        # 4 chunks of x^T
            wq = cpool.tile([128, KC * 128], bf16)
            wk = cpool.tile([128, KC * 128], bf16)
            wv = cpool.tile([128, KC * 128], bf16)
            wouta = cpool.tile([HD, EMBED], bf16)
            woutb = cpool.tile([HD, EMBED], bf16)
            augL = [cpool.tile([128, S], bf16, tag=f"augL{h}", name=f"augL{h}") for h in range(2)]
            augR = [cpool.tile([128, S], bf16, tag=f"augR{h}", name=f"augR{h}") for h in range(2)]
            oc = cpool.tile([48, 384], bf16)
            colr = [cpool.tile([48, S], bf16, tag=f"colr{h}", name=f"colr{h}") for h in range(2)]
            vv = [cpool.tile([128, NM * 65], bf16, tag=f"vv{h}", name=f"vv{h}") for h in range(2)]
            outT = [cpool.tile([65, S], bf16, tag=f"outT{h}", name=f"outT{h}") for h in range(2)]
            rcol = [cpool.tile([128, NT], f32, tag=f"rcol{h}", name=f"rcol{h}") for h in range(2)]
            ident = cpool.tile([128, 128], bf16)

            make_identity(nc, ident[:, :])

            # ---- DMA inputs ----
            for c in range(KC):
                nc.sync.dma_start(out=xT[:, ds(c * S, S)],
                                  in_=xT_d[ts(c, 128), :])
                nc.sync.dma_start(out=wq[:, ts(c, 128)], in_=wq_d[ts(c, 128), :])
                nc.sync.dma_start(out=wk[:, ts(c, 128)], in_=wk_d[ts(c, 128), :])
                nc.sync.dma_start(out=wv[:, ts(c, 128)], in_=wv_d[ts(c, 128), :])
            nc.sync.dma_start(out=wouta[:, :], in_=wouta_d[:, :])
            nc.sync.dma_start(out=woutb[:, :], in_=woutb_d[:, :])
            for h, (rowr_d, colr_d) in enumerate(
                    [(rowra_d, colra_d), (rowrb_d, colrb_d)]):
                nc.sync.dma_start(out=augL[h][64:128, :], in_=ohr_d[:, :])
                nc.sync.dma_start(out=augR[h][64:128, :], in_=rowr_d[:, :])
                nc.sync.dma_start(out=colr[h][:, :], in_=colr_d[:, :])
            nc.sync.dma_start(out=oc[:, :], in_=oc_d[:, :])

            # ones columns of v_aug (overwritten below for cols 0..63)
            nc.vector.memset(vv[0][:, :], 1.0)
            nc.vector.memset(vv[1][:, :], 1.0)

            # ---- projections: qT, kT  (heads separate, M=64) ----
            for h in range(2):
                hofs = 64 * h
                for n in range(NQ):
                    pq = pmisc.tile([64, 512], f32, tag="pm")
                    pk = pmisc.tile([64, 512], f32, tag="pm")
                    for c in range(KC):
                        lw_q = wq[:, ds(c * 128 + hofs, 64)]
                        lw_k = wk[:, ds(c * 128 + hofs, 64)]
                        rx = xT[:, ds(c * S + n * 512, 512)]
                        nc.tensor.matmul(pq[:, :], lw_q, rx,
                                         start=(c == 0), stop=(c == KC - 1))
                        nc.tensor.matmul(pk[:, :], lw_k, rx,
                                         start=(c == 0), stop=(c == KC - 1))
                    nc.vector.tensor_copy(augR[h][0:64, ts(n, 512)], pq[:, :])
                    nc.vector.tensor_copy(augL[h][0:64, ts(n, 512)], pk[:, :])

            # ---- projections: vT then PE-transpose into v natural ----
            for h in range(2):
                hofs = 64 * h
                for n in range(NQ):
                    pv = pmisc.tile([64, 512], f32, tag="pm")
                    for c in range(KC):
                        lw_v = wv[:, ds(c * 128 + hofs, 64)]
                        rx = xT[:, ds(c * S + n * 512, 512)]
                        nc.tensor.matmul(pv[:, :], lw_v, rx,
                                         start=(c == 0), stop=(c == KC - 1))
                    vtw = wpool.tile([64, 512], bf16, tag="vtw", name="vtw")
                    nc.vector.tensor_copy(vtw[:, :], pv[:, :])
                    for mm in range(4):
                        m = n * 4 + mm
                        ptr = pmisc.tile([128, 64], bf16, tag="pm")
                        nc.tensor.transpose(ptr[:, :], vtw[:, ts(mm, 128)],
                                            ident[0:64, 0:64])
                        nc.vector.tensor_copy(vv[h][:, ds(m * 65, 64)], ptr[:, :])

            # ---- attention main loop ----
            for n in range(NQ):
                for h in range(2):
                    acc = pacc.tile([65, 512], f32, tag="acc")
                    for m0 in range(0, NM, 2):
                        st = pst.tile([128, 1024], f32, tag="st")
                        pt = ptpool.tile([128, 1024], bf16, tag="pt")
                        for k in range(2):
                            m = m0 + k
                            sl = ds(k * 512, 512)
                            nc.tensor.matmul(
                                st[:, sl],
                                augL[h][:, ts(m, 128)],
                                augR[h][:, ts(n, 512)],
                                start=True, stop=False)
                            nc.tensor.matmul(
                                st[:, sl],
                                oc[:, ts(m % 3, 128)],
                                colr[h][:, ts(n, 512)],
                                start=False, stop=True)
                        nc.scalar.activation(pt[:, :], st[:, :], EXP)
                        for k in range(2):
                            m = m0 + k
                            nc.tensor.matmul(
                                acc[:, :],
                                vv[h][:, ds(m * 65, 65)],
                                pt[:, ds(k * 512, 512)],
                                start=(m == 0), stop=(m == NM - 1))
                    nc.vector.tensor_copy(outT[h][:, ts(n, 512)], acc[:, :])

            # ---- rowsum -> per-q-partition reciprocal ----
            for h in range(2):
                for t in range(NT):
                    ptr = pmisc.tile([128, 65], bf16, tag="pm")
                    nc.tensor.transpose(ptr[:, :], outT[h][:, ts(t, 128)],
                                        ident[0:65, 0:65])
                    nc.vector.tensor_copy(rcol[h][:, ts(t, 1)],
                                          ptr[:, ds(64, 1)])
                nc.vector.reciprocal(rcol[h][:, :], rcol[h][:, :])

            # ---- output projection + normalize + combine heads ----
            for t in range(NT):
                fpa = pmisc.tile([128, 512], f32, tag="pm")
                fpb = pmisc.tile([128, 512], f32, tag="pm")
                nc.tensor.matmul(fpa[:, :],
                                 outT[0][0:64, ts(t, 128)],
                                 wouta[:, :],
                                 start=True, stop=True)
                nc.tensor.matmul(fpb[:, :],
                                 outT[1][0:64, ts(t, 128)],
                                 woutb[:, :],
                                 start=True, stop=True)
                ta = opool.tile([128, 512], f32, tag="ta")
                tb = opool.tile([128, 512], f32, tag="tb")
                nc.vector.tensor_scalar_mul(ta[:, :], fpa[:, :],
                                            rcol[0][:, ts(t, 1)])
                nc.vector.tensor_scalar_mul(tb[:, :], fpb[:, :],
                                            rcol[1][:, ts(t, 1)])
                osb = opool.tile([128, 512], f32, tag="osb")
                nc.vector.tensor_add(osb[:, :], ta[:, :], tb[:, :])
                nc.sync.dma_start(out=out_d[ts(t, 128), :], in_=osb[:, :])

    nc.compile()
    return nc


def _get_nc():
    if "nc" not in _CACHE:
        _CACHE["nc"] = _build_program()
    return _CACHE["nc"]


def _prep_core_inputs(x, w_qkv, w_out, rel_row_tab, rel_col_tab):
    """Per-core input dicts (host-side shard + constant precompute)."""
    import ml_dtypes
    bf = np.float16
    x = np.asarray(x, np.float32)
    w_qkv = np.asarray(w_qkv, np.float32)
    w_out = np.asarray(w_out, np.float32)
    rel_row_tab = np.asarray(rel_row_tab, np.float32)
    rel_col_tab = np.asarray(rel_col_tab, np.float32)

    ri = np.arange(S) // GW           # grid row of flat index
    ci = np.arange(S) % GW            # grid col of flat index
    ohr = (ri[None, :] == np.arange(64)[:, None]).astype(np.float32)
    oc = (ci[None, :] == np.arange(48)[:, None]).astype(np.float32)
    oc = np.ascontiguousarray(oc[:, :384])
    # rowr[h][t, i] = rel_row_tab[ri[i] - t + 63, h]; idx in [0,126] (no clip)
    row_idx = ri[None, :] - np.arange(64)[:, None] + 63   # [64, S]
    col_idx = ci[None, :] - np.arange(48)[:, None] + 47   # [48, S]

    scale = HD ** -0.5
    in_maps = []
    for c in range(N_CORES):
        b = c // 4
        h0 = 2 * (c % 4)
        h1 = h0 + 1
        xT = np.ascontiguousarray(x[b].reshape(S, EMBED).T)
        def wslice(base, h):
            return w_qkv[:, base + h * HD: base + (h + 1) * HD]
        wq = np.concatenate([wslice(0, h0), wslice(0, h1)], axis=1) * scale
        wk = np.concatenate([wslice(EMBED, h0), wslice(EMBED, h1)], axis=1)
        wv = np.concatenate([wslice(2 * EMBED, h0), wslice(2 * EMBED, h1)],
                            axis=1)
        in_maps.append({
            "xT": xT.astype(bf),
            "wq": np.ascontiguousarray(wq).astype(bf),
            "wk": np.ascontiguousarray(wk).astype(bf),
            "wv": np.ascontiguousarray(wv).astype(bf),
            "wouta": np.ascontiguousarray(w_out[h0 * HD:(h0 + 1) * HD, :]).astype(bf),
            "woutb": np.ascontiguousarray(w_out[h1 * HD:(h1 + 1) * HD, :]).astype(bf),
            "ohr": ohr.astype(bf),
            "rowra": np.ascontiguousarray(rel_row_tab[row_idx, h0]).astype(bf),
            "rowrb": np.ascontiguousarray(rel_row_tab[row_idx, h1]).astype(bf),
            "oc": oc.astype(bf),
            "colra": np.ascontiguousarray(rel_col_tab[col_idx, h0]).astype(bf),
            "colrb": np.ascontiguousarray(rel_col_tab[col_idx, h1]).astype(bf),
        })
    return in_maps


def _run(inputs, trace=False):
    from concourse.bass_utils import run_bass_kernel_spmd
    nc = _get_nc()
    in_maps = _prep_core_inputs(**inputs)
    res = run_bass_kernel_spmd(nc, in_maps, list(range(N_CORES)), trace=trace)
    acc = np.zeros((B, S, EMBED), np.float32)
    for c in range(N_CORES):
        acc[c // 4] += res.results[c]["out"]
    return acc.reshape(B, GH, GW, EMBED), res


def kernel(x, w_qkv, w_out, rel_row_tab, rel_col_tab):
    out, _ = _run(dict(x=x, w_qkv=w_qkv, w_out=w_out,
                       rel_row_tab=rel_row_tab, rel_col_tab=rel_col_tab))
    return out



# revision 2
# speedup vs baseline: 1.7385x; 1.7385x over previous
"""GridAttention Trainium2 kernel.

Full inputs -> full output. Internally shards (batch, head-pair) across 8
NeuronCores: core c handles batch c//4 and heads (2*(c%4), 2*(c%4)+1).

Math notes:
 - Attention scores are computed TRANSPOSED: S^T[j, i] = k_j . q_i * scale
   + rowbias[i, j], laid out [k partitions, q free]. This makes softmax-exp
   elementwise, the denominator a matmul reduction (ones column in V), and
   P^T directly usable as the moving operand of the AV matmul.
 - The 2D relative-position bias splits additively:
     bias[i, j] = rowtab[ri-rj+63, h] + coltab[ci-cj+47, h]
   (no clipping needed since H==MAX_H, W==MAX_W).
   * ROW bias rides inside the QK matmul: the contraction is augmented to
     K=128 = [qk 64 | onehot(rj) 64] against [q 64 | rowr 64]; matmul cost
     on TRN2 is N-columns only, so this is free.
   * COL bias is applied MULTIPLICATIVELY after exp: P = exp(qk+row) *
     exp(colbias). exp(colbias)^T tiles are periodic with period 3 in both
     the 128-wide k-chunk index and the 512-wide q-chunk index (since
     128%48=32, 512%48=32, 3*32%48=0), so only 9 distinct [128, 512]
     blocks exist; they are host-precomputed and the multiply is a single
     DVE tensor_mul per [128, 1536] exp tile.
 - No max-subtraction in softmax: logits ~ N(0,1), exp is safe in fp32/fp16
   and softmax is shift-invariant so results match the reference.
 - Softmax normalization and head combination happen ON HOST: the device
   emits, per head, the UNNORMALIZED projected output (P_h V_h W_h) in fp16
   plus the per-query denominator row; host computes sum_h out_h / d_h.
 - All matmul operands are fp16; PSUM accumulation stays fp32.
"""

import numpy as np

EMBED = 512
NH = 8
HD = 64
GH, GW = 64, 48
B = 2
S = GH * GW  # 3072
N_CORES = 8
NQ = S // 512  # 6 q chunks of 512
NM = S // 128  # 24 k chunks of 128
NG = NM // 3   # 8 groups of 3 k-chunks per (n, h)
KC = 4         # 512 = 4 contraction chunks of 128

_CACHE = {}


def _build_program():
    import concourse.bass as bass
    import concourse.tile as tile
    import concourse.mybir as mybir
    from concourse import bacc
    from concourse.bass import ts, ds
    from concourse.masks import make_identity

    f32 = mybir.dt.float32
    f16 = mybir.dt.float16
    EXP = mybir.ActivationFunctionType.Exp

    nc = bacc.Bacc("TRN2", target_bir_lowering=False, debug=False,
                   num_devices=N_CORES)

    def inp(name, shape):
        return nc.dram_tensor(name, shape, f16, kind="ExternalInput").ap()

    xT_d = inp("xT", [EMBED, S])
    wq_d = inp("wq", [EMBED, 128])
    wk_d = inp("wk", [EMBED, 128])
    wv_d = inp("wv", [EMBED, 128])
    wouta_d = inp("wouta", [HD, EMBED])
    woutb_d = inp("woutb", [HD, EMBED])
    ohr_d = inp("ohr", [64, S])
    rowra_d = inp("rowra", [64, S])
    rowrb_d = inp("rowrb", [64, S])
    ecola_d = inp("ecola", [128, 3 * 1536])
    ecolb_d = inp("ecolb", [128, 3 * 1536])
    outa_d = nc.dram_tensor("outa", [S, EMBED], f16, kind="ExternalOutput").ap()
    outb_d = nc.dram_tensor("outb", [S, EMBED], f16, kind="ExternalOutput").ap()
    den_d = nc.dram_tensor("den", [2, S], f16, kind="ExternalOutput").ap()

    with tile.TileContext(nc) as tc:
        with (
            tc.tile_pool(name="const", bufs=1) as cpool,
            tc.tile_pool(name="vtwp", bufs=2) as vtwp,
            tc.tile_pool(name="ptp", bufs=3) as ptp,
            tc.tile_pool(name="ptmp", bufs=3) as ptmp,
            tc.tile_pool(name="osb", bufs=3) as opool,
            tc.tile_pool(name="ps", bufs=2, space="PSUM") as ps,
        ):
            # ---- resident SBUF tensors ----
            xT = cpool.tile([128, KC * S], f16)
            wq = cpool.tile([128, KC * 128], f16)
            wk = cpool.tile([128, KC * 128], f16)
            wv = cpool.tile([128, KC * 128], f16)
            wout = [cpool.tile([HD, EMBED], f16, tag=f"wout{h}", name=f"wout{h}")
                    for h in range(2)]
            augL = [cpool.tile([128, S], f16, tag=f"augL{h}", name=f"augL{h}")
                    for h in range(2)]
            augR = [cpool.tile([128, S], f16, tag=f"augR{h}", name=f"augR{h}")
                    for h in range(2)]
            ecol = [cpool.tile([128, 3 * 1536], f16, tag=f"ecol{h}",
                               name=f"ecol{h}") for h in range(2)]
            vv = [cpool.tile([128, NM * 65], f16, tag=f"vv{h}", name=f"vv{h}")
                  for h in range(2)]
            outT = [cpool.tile([65, S], f16, tag=f"outT{h}", name=f"outT{h}")
                    for h in range(2)]
            ident = cpool.tile([128, 128], f16)

            make_identity(nc, ident[:, :])

            # ---- DMA inputs (in consumption order) ----
            for c in range(KC):
                nc.sync.dma_start(out=xT[:, ds(c * S, S)],
                                  in_=xT_d[ts(c, 128), :])
            for c in range(KC):
                nc.sync.dma_start(out=wk[:, ts(c, 128)], in_=wk_d[ts(c, 128), :])
                nc.sync.dma_start(out=wv[:, ts(c, 128)], in_=wv_d[ts(c, 128), :])
                nc.sync.dma_start(out=wq[:, ts(c, 128)], in_=wq_d[ts(c, 128), :])
            for h, (rowr_d, ecol_d) in enumerate(
                    [(rowra_d, ecola_d), (rowrb_d, ecolb_d)]):
                nc.sync.dma_start(out=augL[h][64:128, :], in_=ohr_d[:, :])
                nc.sync.dma_start(out=augR[h][64:128, :], in_=rowr_d[:, :])
                nc.sync.dma_start(out=ecol[h][:, :], in_=ecol_d[:, :])
            nc.sync.dma_start(out=wout[0][:, :], in_=wouta_d[:, :])
            nc.sync.dma_start(out=wout[1][:, :], in_=woutb_d[:, :])

            # ones columns of v_aug (overwritten below for cols 0..63)
            nc.vector.memset(vv[0][:, :], 1.0)
            nc.vector.memset(vv[1][:, :], 1.0)

            # ---- phase 1a: k projection (both heads packed, M=128) ----
            # pk partitions: [head0 kdims 64 | head1 kdims 64]
            for n in range(NQ):
                pk = ps.tile([128, 512], f32, tag="st", name="pk")
                for c in range(KC):
                    nc.tensor.matmul(pk[:, :], wk[:, ts(c, 128)],
                                     xT[:, ds(c * S + n * 512, 512)],
                                     start=(c == 0), stop=(c == KC - 1))
                # PSUM->SBUF evacuation on the (otherwise idle) scalar engine
                nc.scalar.copy(augL[0][0:64, ts(n, 512)], pk[0:64, :])
                nc.scalar.copy(augL[1][0:64, ts(n, 512)], pk[64:128, :])

            # ---- phase 1b: v projection + PE-transpose into k-natural ----
            for n in range(NQ):
                pv = ps.tile([128, 512], f32, tag="acc", name="pv")
                for c in range(KC):
                    nc.tensor.matmul(pv[:, :], wv[:, ts(c, 128)],
                                     xT[:, ds(c * S + n * 512, 512)],
                                     start=(c == 0), stop=(c == KC - 1))
                vtw = vtwp.tile([128, 512], f16, tag="vtw", name="vtw")
                nc.scalar.copy(vtw[:, :], pv[:, :])
                for mm in range(4):
                    m = n * 4 + mm
                    ptr = ps.tile([128, 128], f16, tag="acc", name="ptr")
                    nc.tensor.transpose(ptr[:, :], vtw[:, ts(mm, 128)],
                                        ident[:, :])
                    nc.vector.tensor_copy(vv[0][:, ds(m * 65, 64)],
                                          ptr[:, 0:64])
                    nc.vector.tensor_copy(vv[1][:, ds(m * 65, 64)],
                                          ptr[:, 64:128])

            # ---- phase 1c: q projection for chunk n (emitted lazily) ----
            def emit_qproj(n):
                pq = ps.tile([128, 512], f32, tag="st", name="pq")
                for c in range(KC):
                    nc.tensor.matmul(pq[:, :], wq[:, ts(c, 128)],
                                     xT[:, ds(c * S + n * 512, 512)],
                                     start=(c == 0), stop=(c == KC - 1))
                nc.scalar.copy(augR[0][0:64, ts(n, 512)], pq[0:64, :])
                nc.scalar.copy(augR[1][0:64, ts(n, 512)], pq[64:128, :])

            emit_qproj(0)
            emit_qproj(1)

            # ---- phase 3 (emitted interleaved): output proj for chunk n ----
            def emit_tail(n):
                for tt in range(4):
                    t = n * 4 + tt
                    for h in range(2):
                        fp = ps.tile([128, 512], f32, tag="acc", name="fp")
                        nc.tensor.matmul(fp[:, :],
                                         outT[h][0:64, ts(t, 128)],
                                         wout[h][:, :],
                                         start=True, stop=True)
                        osb = opool.tile([128, 512], f16, tag="osb",
                                         name="osb")
                        nc.vector.tensor_copy(osb[:, :], fp[:, :])
                        out_d = outa_d if h == 0 else outb_d
                        nc.sync.dma_start(out=out_d[ts(t, 128), :],
                                          in_=osb[:, :])

            # ---- phase 2: attention main loop (software-pipelined by 1) ----
            # stream of groups: (n, h, g); emit scores(i+1) before AV(i) so
            # the in-order PE queue never waits on exp/mul of the same group.
            groups = [(n, h, g) for n in range(NQ) for h in range(2)
                      for g in range(NG)]

            def emit_scores(i):
                n, h, g = groups[i]
                st = ps.tile([128, 1536], f32, tag="st", name="st")
                for k in range(3):
                    m = 3 * g + k
                    nc.tensor.matmul(st[:, ts(k, 512)],
                                     augL[h][:, ts(m, 128)],
                                     augR[h][:, ts(n, 512)],
                                     start=True, stop=True)
                return st

            acc = {}
            pt_of = {}

            def emit_expmul(i):
                n, h, g = groups[i]
                st = pt_of.pop(("st", i))
                pt = ptp.tile([128, 1536], f16, tag="pt", name="pt")
                nc.scalar.activation(pt[:, :], st[:, :], EXP)
                ptm = ptmp.tile([128, 1536], f16, tag="ptm", name="ptm")
                nc.vector.tensor_mul(ptm[:, :], pt[:, :],
                                     ecol[h][:, ds((n % 3) * 1536, 1536)])
                pt_of[("ptm", i)] = ptm

            def emit_av(i):
                n, h, g = groups[i]
                ptm = pt_of.pop(("ptm", i))
                if g == 0:
                    acc[(n, h)] = ps.tile([65, 512], f32, tag="acc",
                                          name="acc")
                a = acc[(n, h)]
                for k in range(3):
                    m = 3 * g + k
                    nc.tensor.matmul(a[:, :],
                                     vv[h][:, ds(m * 65, 65)],
                                     ptm[:, ts(k, 512)],
                                     start=(m == 0), stop=(m == NM - 1))
                if g == NG - 1:
                    nc.vector.tensor_copy(outT[h][:, ts(n, 512)], a[:, :])
                    del acc[(n, h)]

            NGRP = len(groups)
            for i in range(NGRP):
                pt_of[("st", i)] = emit_scores(i)
                emit_expmul(i)
                if i >= 1:
                    emit_av(i - 1)
                n, h, g = groups[i]
                # lazy q-proj: before starting (n, h=1), project q chunk n+2
                if h == 1 and g == 0 and n + 2 < NQ:
                    emit_qproj(n + 2)
                # tail for chunk n-1 once (n, h=0) is fully emitted (its outT
                # copies are long since retired -> no PE stall)
                if h == 1 and g == 0 and n >= 1:
                    emit_tail(n - 1)
            emit_av(NGRP - 1)
            emit_tail(NQ - 1)

            # denominator rows (row 64 of outT = sum_k P)
            nc.sync.dma_start(out=den_d[0:1, :], in_=outT[0][64:65, :])
            nc.sync.dma_start(out=den_d[1:2, :], in_=outT[1][64:65, :])

    nc.compile()
    return nc


def _get_nc():
    if "nc" not in _CACHE:
        _CACHE["nc"] = _build_program()
    return _CACHE["nc"]


def _prep_core_inputs(x, w_qkv, w_out, rel_row_tab, rel_col_tab):
    """Per-core input dicts (host-side shard + constant precompute)."""
    bf = np.float16
    x = np.asarray(x, np.float32)
    w_qkv = np.asarray(w_qkv, np.float32)
    w_out = np.asarray(w_out, np.float32)
    rel_row_tab = np.asarray(rel_row_tab, np.float32)
    rel_col_tab = np.asarray(rel_col_tab, np.float32)

    ri = np.arange(S) // GW           # grid row of flat index
    ci = np.arange(S) % GW            # grid col of flat index
    ohr = (ri[None, :] == np.arange(64)[:, None]).astype(np.float32)
    # rowr[h][t, i] = rel_row_tab[ri[i] - t + 63, h]; idx in [0,126] (no clip)
    row_idx = ri[None, :] - np.arange(64)[:, None] + 63   # [64, S]

    # ecol[h]: 9 periodic blocks of exp(colbias^T).
    # Block (q3=n%3, p=m%3): [jj, ii] = exp(coltab[(32*q3+ii)%48 - (32*p+jj)%48 + 47])
    jj = np.arange(128)
    ii = np.arange(512)
    ecol_idx = np.zeros((3, 3, 128, 512), np.int64)
    for q3 in range(3):
        for p in range(3):
            cj = (32 * p + jj) % 48
            c_i = (32 * q3 + ii) % 48
            ecol_idx[q3, p] = c_i[None, :] - cj[:, None] + 47
    # layout: [128, q3 * 1536 + p * 512 + ii]
    ecol_idx = ecol_idx.transpose(2, 0, 1, 3).reshape(128, 3 * 1536)

    scale = HD ** -0.5
    in_maps = []
    for c in range(N_CORES):
        b = c // 4
        h0 = 2 * (c % 4)
        h1 = h0 + 1
        xT = np.ascontiguousarray(x[b].reshape(S, EMBED).T)
        def wslice(base, h):
            return w_qkv[:, base + h * HD: base + (h + 1) * HD]
        wq = np.concatenate([wslice(0, h0), wslice(0, h1)], axis=1) * scale
        wk = np.concatenate([wslice(EMBED, h0), wslice(EMBED, h1)], axis=1)
        wv = np.concatenate([wslice(2 * EMBED, h0), wslice(2 * EMBED, h1)],
                            axis=1)
        in_maps.append({
            "xT": xT.astype(bf),
            "wq": np.ascontiguousarray(wq).astype(bf),
            "wk": np.ascontiguousarray(wk).astype(bf),
            "wv": np.ascontiguousarray(wv).astype(bf),
            "wouta": np.ascontiguousarray(w_out[h0 * HD:(h0 + 1) * HD, :]).astype(bf),
            "woutb": np.ascontiguousarray(w_out[h1 * HD:(h1 + 1) * HD, :]).astype(bf),
            "ohr": ohr.astype(bf),
            "rowra": np.ascontiguousarray(rel_row_tab[row_idx, h0]).astype(bf),
            "rowrb": np.ascontiguousarray(rel_row_tab[row_idx, h1]).astype(bf),
            "ecola": np.exp(rel_col_tab[ecol_idx, h0]).astype(bf),
            "ecolb": np.exp(rel_col_tab[ecol_idx, h1]).astype(bf),
        })
    return in_maps


def _run(inputs, trace=False):
    from concourse.bass_utils import run_bass_kernel_spmd
    nc = _get_nc()
    in_maps = _prep_core_inputs(**inputs)
    res = run_bass_kernel_spmd(nc, in_maps, list(range(N_CORES)), trace=trace)
    acc = np.zeros((B, S, EMBED), np.float32)
    for c in range(N_CORES):
        r = res.results[c]
        den = np.asarray(r["den"], np.float32)          # [2, S]
        acc[c // 4] += np.asarray(r["outa"], np.float32) / den[0][:, None]
        acc[c // 4] += np.asarray(r["outb"], np.float32) / den[1][:, None]
    return acc.reshape(B, GH, GW, EMBED), res


def kernel(x, w_qkv, w_out, rel_row_tab, rel_col_tab):
    out, _ = _run(dict(x=x, w_qkv=w_qkv, w_out=w_out,
                       rel_row_tab=rel_row_tab, rel_col_tab=rel_col_tab))
    return out
